# revision 10
# baseline (speedup 1.0000x reference)
"""Env-specific MLP heads on 8 trn2 cores.

out[i] = Linear2(relu(Linear1(h[i]))) using the weights of head env_ids[i].

Strategy (expert-parallel with host-side routing):
  - Host sorts tokens by env id. Env e's tokens are split between cores
    2e and 2e+1 (E=4 envs, 8 cores), zero-padded to a common length T.
  - Each core runs a dense 2-layer MLP on its [T, D] token block with a
    single env's weights: no masking, no wasted env compute.
  - Activations live in transposed [feature, token] layout on-chip, so
    both matmuls use natural-layout weight tiles as the stationary
    operand and biases are per-partition ACT-engine bias adds.

PE-efficiency notes (this revision):
  - The token axis T=1024 is processed as two 512-wide chunks (PSUM bank
    width).  For every stationary weight tile the two chunks' matmuls are
    emitted back-to-back, and a post-scheduling pass removes the second
    (redundant) Ldweights the tile scheduler inserts — halving stationary
    loads, which are not overlapped with streaming on TRN2.
  - All PE matmuls are chained with no-sync deps so the scheduled program
    order keeps the pairs adjacent (required for the Ldweights dedupe).
  - w1 and xt are packed by the host in DRAM in the exact SBUF layout
    ([P, h, k, c] / [P, chunk, k, c]) so every DMA moves contiguous
    >=2KB lines on both sides; w1 streams in column stripes so the first
    L1 group starts after ~0.5 MB instead of 4 MB.
  - The first two L1 h-groups run chunk0-only then chunk1-only (needs
    only xt chunk0 + one stripe to start); everything later is paired.
  - The output is DMA'd as bf16 (host upcasts); well within tolerance
    and halves output traffic.
"""

import numpy as np
import ml_dtypes

import concourse.mybir as mybir
import concourse.tile as tile
from concourse import bacc
from concourse.bass_utils import run_bass_kernel_spmd
from concourse.tile import add_dep_helper

P = 128
NCORES = 8
CN = 512  # fp32 PSUM bank width = moving-dim chunk
N_WARM = 16


def _dedupe_ldweights(nc):
    """Remove Ldweights that reload the stationary tile already in the PE
    array.  The tile scheduler emits one Ldweights per matmul
    unconditionally; consecutive matmuls sharing lhsT only need the first.
    Only removes loads that carry no semaphore waits/updates and follow an
    identical load with nothing but ldweights=False matmuls in between."""
    ndel = 0
    for blk in nc.main_func.blocks:
        cur = None
        todel = []
        for ins in blk.instructions:
            if ins.engine != mybir.EngineType.PE:
                continue
            if isinstance(ins, mybir.InstLdweights):
                si = ins.sync_info
                busy = si is not None and (
                    len(si.on_wait) > 0 or len(si.on_update) > 0
                )
                sig = (repr(ins.ins[0]), str(ins.perf_mode), str(ins.is_transpose))
                if cur == sig and not busy:
                    todel.append(ins)
                else:
                    cur = sig
            elif isinstance(ins, mybir.InstMatmult):
                if ins.ldweights is not False:
                    cur = None
            else:
                cur = None
        for ins in todel:
            blk.instructions.remove(ins)
        ndel += len(todel)
    return ndel


def build_nc(T, D=1024, Hdim=2048, A=1024, iters=1, n_warm=N_WARM, dedupe=True,
             pair=True, out_bf16=True):
    """Bass program for one core: out[A,T] = W2.T@relu(W1.T@xt + b1) + b2.

    iters>1 repeats the compute phase (for steady-state HW timing only).
    """
    KO1, KO2, AT = D // P, Hdim // P, A // P
    bf16, f32 = mybir.dt.bfloat16, mybir.dt.float32
    out_dt = bf16 if out_bf16 else f32
    assert T % CN == 0
    NCH = T // CN
    chunks = [(i * CN, CN) for i in range(NCH)]

    nc = bacc.Bacc(
        "TRN2", target_bir_lowering=False, debug=True, num_devices=NCORES,
    )

    # DRAM layouts match the SBUF tiles exactly (host packs them), so every
    # DMA is a contiguous >=2KB-line copy on both sides.
    xt = nc.dram_tensor("xt", [P, NCH, KO1, CN], bf16, kind="ExternalInput")
    w1 = nc.dram_tensor("w1", [P, KO2, KO1, P], bf16, kind="ExternalInput")
    b1 = nc.dram_tensor("b1", [P, KO2], f32, kind="ExternalInput")
    w2 = nc.dram_tensor("w2", [Hdim, A], bf16, kind="ExternalInput")
    b2 = nc.dram_tensor("b2", [P, AT], f32, kind="ExternalInput")
    out = nc.dram_tensor("out", [A, T], out_dt, kind="ExternalOutput")

    with tile.TileContext(nc) as tc:
        with (
            tc.tile_pool(name="weights", bufs=1) as wp,
            tc.tile_pool(name="acts", bufs=1) as acts,
            tc.tile_pool(name="ps1", bufs=4, space="PSUM") as pp1,
            tc.tile_pool(name="ps2", bufs=4, space="PSUM") as pp2,
            tc.tile_pool(name="outs", bufs=4) as op,
        ):
            w1_sb = wp.tile([P, KO2, KO1, P], bf16, tag="w1")
            w2_sb = wp.tile([P, KO2, A], bf16, tag="w2")
            b1_sb = wp.tile([P, KO2], f32, tag="b1")
            b2_sb = wp.tile([P, AT], f32, tag="b2")
            xt_sb = acts.tile([P, NCH, KO1, CN], bf16, tag="xt")
            hid_sb = acts.tile([P, KO2, T], bf16, tag="hid")

            # --- DMA plan ---
            # Head-critical transfers first on every ring: the unpaired
            # head (h=0,1 on chunk0) needs stripe h0 + xt chunk0 + b1 only.
            # Scalar ring stays short (it also runs every activation).
            nc.scalar.dma_start(w1_sb[:, 0], w1[:, 0])
            nc.scalar.dma_start(w1_sb[:, 1], w1[:, 1])
            nc.scalar.dma_start(b1_sb[:], b1[:])
            nc.scalar.dma_start(b2_sb[:], b2[:])
            # xt chunk0 (split in k-halves so the first psum group can
            # start after 512KB) on the SP ring, chunk1 on the gpsimd
            # SWDGE queue — in parallel during the warmup matmuls.
            Kh = KO1 // 2
            nc.sync.dma_start(xt_sb[:, 0, :Kh], xt[:, 0, :Kh])
            nc.sync.dma_start(xt_sb[:, 0, Kh:], xt[:, 0, Kh:])
            for ci in range(1, NCH):
                nc.gpsimd.dma_start(xt_sb[:, ci], xt[:, ci])
            # Remaining w1 stripes stream on the SP ring in pairs (fewer
            # ring triggers), in the order the L1 h-groups consume them.
            last_w1 = None
            for h in range(2, KO2, 2):
                last_w1 = nc.sync.dma_start(w1_sb[:, h : h + 2], w1[:, h : h + 2])
            # w2 in 4 large DMAs split gpsimd/SP; gate the first SWDGE one
            # behind the w1 stripes (shared SDMA pool — an eager w2 would
            # starve the PE's startup inputs).
            K4 = KO2 // 4
            for j in range(4):
                eng = nc.gpsimd if j % 2 == 0 else nc.sync
                w2_dma = eng.dma_start(
                    w2_sb[:, j * K4 : (j + 1) * K4, :],
                    w2[j * K4 * P : (j + 1) * K4 * P, :].rearrange(
                        "(ko p) c -> p ko c", p=P
                    ),
                )
                if j % 2 == 0 and last_w1 is not None:
                    add_dep_helper(
                        w2_dma.ins, last_w1.ins, sync=True,
                        reason="defer w2 SWDGE behind w1 stripes",
                    )

            # PE pre-warm: the HAM clock-gate starts at half clock and needs
            # ~3.4us of sustained PE activity to unthrottle; run dummy
            # matmuls on memset scratch while the input prefix lands.
            prev_mm = [None]

            def chain(ins):
                if prev_mm[0] is not None:
                    add_dep_helper(ins.ins, prev_mm[0].ins, sync=False,
                                   reason="pe chain")
                prev_mm[0] = ins
                return ins

            if n_warm:
                wsc = wp.tile([P, P], bf16, tag="warm_w")
                xsc = wp.tile([P, CN], bf16, tag="warm_x")
                nc.vector.memset(wsc[:], 0.0)
                nc.vector.memset(xsc[:], 0.0)
                ps_warm = pp2.tile([P, CN], f32, tag="ps2", name="ps_warm")
                for _ in range(n_warm):
                    chain(nc.tensor.matmul(ps_warm[:], wsc[:], xsc[:],
                                           start=True, stop=True))

            def l1_relu(h, ci, ps):
                t0, tn = chunks[ci]
                nc.scalar.activation(
                    hid_sb[:, h, t0 : t0 + tn],
                    ps[:],
                    mybir.ActivationFunctionType.Relu,
                    bias=b1_sb[:, h : h + 1],
                )

            def l1_group(h, chunk_ids):
                pss = {ci: pp1.tile([P, chunks[ci][1]], f32, tag="ps1", name="ps1")
                       for ci in chunk_ids}
                for k in range(KO1):
                    for ci in chunk_ids:
                        chain(nc.tensor.matmul(
                            pss[ci][:],
                            w1_sb[:, h, k],
                            xt_sb[:, ci, k],
                            start=(k == 0),
                            stop=(k == KO1 - 1),
                        ))
                for ci in chunk_ids:
                    l1_relu(h, ci, pss[ci])

            def l1_head():
                # The first four (h, chunk) groups, ordered by when their
                # inputs land: h0/h1 on chunk0 k-low first (needs only the
                # first xt-chunk0 half and stripes h0/h1), then their
                # k-high halves, then the chunk1 twins — so the PE has
                # work in flight while the rest of xt is still in transit.
                Kh = KO1 // 2
                gs = [(0, 0), (1, 0), (0, 1), (1, 1)]
                pss = {g: pp1.tile([P, chunks[g[1]][1]], f32, tag="ps1", name="ps1")
                       for g in gs}
                passes = [
                    [(0, 0, range(Kh))],
                    [(1, 0, range(Kh))],
                    [(0, 0, range(Kh, KO1)), (1, 0, range(Kh, KO1))],
                    [(0, 1, range(KO1)), (1, 1, range(KO1))],
                ]
                for p in passes:
                    for hg, cg, ks in p:
                        for k in ks:
                            chain(nc.tensor.matmul(
                                pss[(hg, cg)][:],
                                w1_sb[:, hg, k],
                                xt_sb[:, cg, k],
                                start=(k == 0),
                                stop=(k == KO1 - 1),
                            ))
                for g in gs:
                    l1_relu(g[0], g[1], pss[g])

            def l2_group(a, last=False):
                pss = [pp2.tile([P, tn], f32, tag="ps2", name="ps2")
                       for _, tn in chunks]
                for k in range(KO2):
                    for ci, (t0, tn) in enumerate(chunks):
                        chain(nc.tensor.matmul(
                            pss[ci][:],
                            w2_sb[:, k, a * P : (a + 1) * P],
                            hid_sb[:, k, t0 : t0 + tn],
                            start=(k == 0),
                            stop=(k == KO2 - 1),
                        ))
                for ci, (t0, tn) in enumerate(chunks):
                    ot = op.tile([P, tn], out_dt, tag="ot", name="ot")
                    nc.scalar.activation(
                        ot[:],
                        pss[ci][:],
                        mybir.ActivationFunctionType.Identity,
                        bias=b2_sb[:, a : a + 1],
                    )
                    if last:
                        out_eng = nc.gpsimd if ci == 0 else nc.sync
                    else:
                        out_eng = nc.gpsimd
                    out_eng.dma_start(out[a * P : (a + 1) * P, t0 : t0 + tn], ot[:])

            for it in range(iters):
                if pair and it == 0 and NCH == 2 and KO2 >= 2:
                    l1_head()
                    h0 = 2
                else:
                    h0 = 0
                for h in range(h0, KO2):
                    if pair:
                        l1_group(h, list(range(NCH)))
                    else:
                        for ci in range(NCH):
                            l1_group(h, [ci])
                for a in range(AT):
                    l2_group(a, last=(a == AT - 1))

    if dedupe:
        build_nc.n_deduped = _dedupe_ldweights(nc)
    nc.compile()
    return nc


def make_in_maps(h, env_ids, W1, b1, W2, b2):
    """Route tokens to cores.

    T is fixed at 1024 so the device kernel is two clean 512-wide chunks
    with no inefficient remainder matmuls. Each env gets 2 cores (2048
    token capacity); the few tokens beyond that for over-represented envs
    go to `overflow` and are computed on the host in fp32.

    w1 and xt are packed in the SBUF tile layout ([P, h, k, c] /
    [P, chunk, k, c]) so device DMAs are contiguous on both sides.

    Returns (in_maps, core_tokens, overflow, T).
    """
    bf16 = ml_dtypes.bfloat16
    B, D = h.shape
    E, _, Hdim = W1.shape
    A = W2.shape[-1]
    KO1, KO2 = D // P, Hdim // P
    cpe = NCORES // E  # cores per env
    assert cpe * E == NCORES
    T = 1024
    NCH = T // CN

    env = np.asarray(env_ids).reshape(-1).astype(np.int64)
    order = np.argsort(env, kind="stable")
    counts = np.bincount(env, minlength=E)
    starts = np.concatenate([[0], np.cumsum(counts)])

    in_maps = []
    core_tokens = []
    overflow = []  # (env, token index array)
    for e in range(E):
        idx = order[starts[e] : starts[e + 1]]
        if len(idx) > cpe * T:
            overflow.append((e, idx[cpe * T :]))
            idx = idx[: cpe * T]
        parts = np.array_split(idx, cpe)
        # [D, Hdim] = [(k p), (h c)] -> [p, h, k, c]
        w1e = np.ascontiguousarray(
            W1[e].astype(bf16).reshape(KO1, P, KO2, P).transpose(1, 2, 0, 3)
        )
        w2e = np.ascontiguousarray(W2[e]).astype(bf16)
        b1e = np.ascontiguousarray(
            b1[e].astype(np.float32).reshape(Hdim // P, P).T
        )
        b2e = np.ascontiguousarray(b2[e].astype(np.float32).reshape(A // P, P).T)
        for s in range(cpe):
            tok = parts[s]
            xt = np.zeros((D, T), dtype=bf16)
            if len(tok):
                xt[:, : len(tok)] = h[tok].astype(bf16).T
            # [D, T] = [(k p), (ci n)] -> [p, ci, k, n]
            xtp = np.ascontiguousarray(
                xt.reshape(KO1, P, NCH, CN).transpose(1, 2, 0, 3)
            )
            in_maps.append({"xt": xtp, "w1": w1e, "b1": b1e, "w2": w2e, "b2": b2e})
            core_tokens.append(tok)
    return in_maps, core_tokens, overflow, T


def kernel(h, env_ids, W1, b1, W2, b2):
    h = np.asarray(h, dtype=np.float32)
    W1 = np.asarray(W1, dtype=np.float32)
    b1 = np.asarray(b1, dtype=np.float32)
    W2 = np.asarray(W2, dtype=np.float32)
    b2 = np.asarray(b2, dtype=np.float32)

    in_maps, core_tokens, overflow, T = make_in_maps(h, env_ids, W1, b1, W2, b2)
    nc = build_nc(T, D=h.shape[1], Hdim=W1.shape[2], A=W2.shape[2])
    res = run_bass_kernel_spmd(nc, in_maps, list(range(NCORES))).results

    B = h.shape[0]
    A = W2.shape[2]
    out = np.zeros((B, A), dtype=np.float32)
    for c in range(NCORES):
        tok = core_tokens[c]
        if len(tok):
            out[tok] = res[c]["out"][:, : len(tok)].T.astype(np.float32)
    for e, tok in overflow:
        hid = np.maximum(h[tok] @ W1[e] + b1[e], 0.0)
        out[tok] = hid @ W2[e] + b2[e]
    return out
